# revision 9
# baseline (speedup 1.0000x reference)
"""Trainium2 Bass kernel for the aperiodic real-space Ewald sum (N=4096).

Math: with w_ij = erf(d_ij/sqrt(2)) / (d_ij + eps) (symmetric),
    t_i   = sum_j q_j w_ij
    field = t/(2*pi) + 2*SELF_C*q
    pot   = (q . t)/(4*pi) + SELF_C*sum(q^2)

Sharding: core c owns rows [c*512, (c+1)*512). Each 128-row chunk computes
its [128, 4096] block of w against ALL columns j, multiplies by q_j along
the free axis, and reduces along the free axis (DVE accum_out) -- giving
t for its own rows directly. No partition reduction and no cross-core
combining; the host concatenates row segments.

d^2 = s_i + s_j - 2 x_i.x_j is computed as a K=13 bf16 matmul using a
double-bf16 (hi+lo) decomposition of both factors, which matches fp32
accuracy (~1e-3 abs) at bf16 PE speed (1 cycle/row vs 4 for fp32):
  rows 0-2: ahi.bhi, 3-5: ahi.blo, 6-8: alo.bhi  (a=-2x_i, b=x_j)
  rows 9,10: (shi+slo)_i * 1,  rows 11,12: 1 * (shi+slo)_j
plus a bf16 BIG*I matmul on the diagonal block so the self-pair lands at
d=2^20 (w_ii ~ 2^-20 ~ 0).

Per chunk: ACT: d = Sqrt(p) [, r = AbsRsqrt(p)]; conv = Erf(d/sqrt(2))
           DVE: [r = reciprocal_approx_fast(d);] u = conv*qb;
                z = u*r (or u/d) with accum_out -> tloc[:, chunk]
"""
import sys

sys.path.insert(0, "/opt/trn_rl_repo")

import numpy as np
import ml_dtypes

import concourse.bass as bass
import concourse.tile as tile
from concourse import bacc, mybir
from concourse.bass_utils import run_bass_kernel_spmd

N = 4096
NCORES = 8
R = N // NCORES          # rows per core
CH = R // 128            # 128-row chunks per core
K = 13                   # contraction depth of the double-bf16 d^2 matmul
SIGMA = 1.0
TWOPI = 2.0 * np.pi
SELF_C = 1.0 / (SIGMA * TWOPI**1.5)
INV_SQRT2 = float(1.0 / np.sqrt(2.0))
BIG = float(2.0**40)
DCLAMP = 0.02            # d~2 = d^2 + DCLAMP: keeps Sqrt off negative inputs
                         # (double-bf16 d^2 err ~ +-0.009, true min d^2 ~ 2e-6;
                         # w(sqrt(d^2+c)) - w(d) <= ~3e-4 since w is flat at 0)

_nc_cache = None


def _build_nc(loop_n=None, unroll=1, *, r_mode="act"):
    """r_mode: 'act'  -> r = AbsRsqrt(p) on ACT (3 ACT passes, 2 DVE)
               'recip'-> r = reciprocal_approx_fast(d) on DVE (2 ACT, 3 DVE)
               'div'  -> z = u / d on DVE (2 ACT, 2 DVE)"""
    nc = bacc.Bacc("TRN2", target_bir_lowering=False, debug=False,
                   num_devices=NCORES)
    f32 = mybir.dt.float32
    bf16 = mybir.dt.bfloat16
    E = mybir.ActivationFunctionType
    mult = mybir.AluOpType.mult
    div = mybir.AluOpType.divide

    aug_d = nc.dram_tensor("aug", [K, R + N], bf16, kind="ExternalInput").ap()
    qb_d = nc.dram_tensor("qb", [128, N], f32, kind="ExternalInput").ap()
    idn_d = nc.dram_tensor("idn", [128, 256], bf16, kind="ExternalInput").ap()
    t_d = nc.dram_tensor("t", [128, CH], f32, kind="ExternalOutput").ap()

    with tile.TileContext(nc) as tc:
        with (
            tc.tile_pool(name="sbin", bufs=1) as sbin,
            tc.tile_pool(name="psum", bufs=1, space="PSUM") as psum,
            tc.tile_pool(name="sbd", bufs=2) as sbd,
            tc.tile_pool(name="sbr", bufs=2) as sbr,
            tc.tile_pool(name="sbc", bufs=2) as sbc,
            tc.tile_pool(name="sbu", bufs=1) as sbu,
            tc.tile_pool(name="sbz", bufs=1) as sbz,
            tc.tile_pool(name="sbt", bufs=2) as sbt,
        ):
            aug = sbin.tile([K, R + N], bf16, tag="aug")
            nc.sync.dma_start(aug[:], aug_d[:])
            qb = sbin.tile([128, N], f32, tag="qb")
            nc.sync.dma_start(qb[:], qb_d[:])
            idn = sbin.tile([128, 256], bf16, tag="idn")
            nc.sync.dma_start(idn[:], idn_d[:])
            cb = sbin.tile([128, 1], f32, tag="cb")
            nc.gpsimd.memset(cb[:], DCLAMP)

            def body():
                tloc = sbt.tile([128, CH], f32, tag="tloc")
                for ic in range(CH):
                    p = psum.tile([128, N], f32, tag="p")
                    for k4 in range(N // 512):
                        nc.tensor.matmul(
                            p[:, k4 * 512:(k4 + 1) * 512],
                            aug[:, ic * 128:(ic + 1) * 128],
                            aug[:, R + k4 * 512:R + (k4 + 1) * 512],
                            start=True, stop=not (k4 == 0))
                        if k4 == 0:
                            nc.tensor.matmul(
                                p[:, ic * 128:(ic + 1) * 128],
                                idn[:, 0:128], idn[:, 128:256],
                                start=False, stop=True)
                    d = sbd.tile([128, N], f32, tag="d")
                    nc.scalar.activation(d[:], p[:], E.Sqrt, bias=cb[:])
                    if r_mode == "act":
                        r = sbr.tile([128, N], f32, tag="r")
                        nc.scalar.activation(r[:], p[:],
                                             E.Abs_reciprocal_sqrt,
                                             bias=cb[:])
                    conv = sbc.tile([128, N], f32, tag="conv")
                    nc.scalar.activation(conv[:], d[:], E.Erf,
                                         scale=INV_SQRT2)
                    if r_mode == "recip":
                        r = sbr.tile([128, N], f32, tag="r")
                        nc.vector.reciprocal_approx_fast(r[:], d[:])
                    u = sbu.tile([128, N], f32, tag="u")
                    nc.vector.tensor_tensor(u[:], conv[:], qb[:], op=mult)
                    z = sbz.tile([128, N], f32, tag="z")
                    if r_mode == "div":
                        nc.vector.scalar_tensor_tensor(
                            out=z[:], in0=u[:], scalar=1.0, in1=d[:],
                            op0=mult, op1=div,
                            accum_out=tloc[:, ic:ic + 1])
                    else:
                        nc.vector.scalar_tensor_tensor(
                            out=z[:], in0=u[:], scalar=1.0, in1=r[:],
                            op0=mult, op1=mult,
                            accum_out=tloc[:, ic:ic + 1])
                nc.sync.dma_start(t_d[:], tloc[:])

            if loop_n is not None:
                with tc.For_i(0, loop_n, 1):
                    for _ in range(unroll):
                        body()
            else:
                body()
    nc.compile()
    return nc


def _hi_lo(v):
    hi = v.astype(ml_dtypes.bfloat16)
    lo = (v - hi.astype(np.float32)).astype(ml_dtypes.bfloat16)
    return hi, lo


def _prep_inputs(positions, q):
    pos = np.ascontiguousarray(np.asarray(positions, dtype=np.float32))
    qv = np.asarray(q, dtype=np.float32).reshape(-1)
    s = (pos * pos).sum(axis=1, dtype=np.float32)

    bhi, blo = _hi_lo(pos.T)              # [3, N]
    shi, slo = _hi_lo(s)                  # [N]
    ahi, alo = _hi_lo(-2.0 * pos.T)       # [3, N]

    rhs_all = np.zeros((K, N), ml_dtypes.bfloat16)
    rhs_all[0:3] = bhi
    rhs_all[3:6] = blo
    rhs_all[6:9] = bhi
    rhs_all[9] = 1.0
    rhs_all[10] = 1.0
    rhs_all[11] = shi
    rhs_all[12] = slo

    lhs_all = np.zeros((K, N), ml_dtypes.bfloat16)
    lhs_all[0:3] = ahi
    lhs_all[3:6] = ahi
    lhs_all[6:9] = alo
    lhs_all[9] = shi
    lhs_all[10] = slo
    lhs_all[11] = 1.0
    lhs_all[12] = 1.0

    idn = np.concatenate([np.eye(128), np.eye(128) * BIG],
                         axis=1).astype(ml_dtypes.bfloat16)

    in_maps = []
    for c in range(NCORES):
        blk = slice(c * R, (c + 1) * R)
        aug = np.empty((K, R + N), ml_dtypes.bfloat16)
        aug[:, 0:R] = lhs_all[:, blk]
        aug[:, R:] = np.roll(rhs_all, -c * R, axis=1)
        qb = np.ascontiguousarray(
            np.broadcast_to(np.roll(qv, -c * R)[None, :], (128, N)),
            dtype=np.float32)
        in_maps.append({"aug": aug, "qb": qb, "idn": idn})
    return in_maps, qv


def kernel(positions, q):
    global _nc_cache
    if _nc_cache is None:
        _nc_cache = _build_nc()
    nc = _nc_cache

    in_maps, qv = _prep_inputs(positions, q)
    res = run_bass_kernel_spmd(nc, in_maps, core_ids=list(range(NCORES)))

    t = np.empty(N, np.float64)
    for c in range(NCORES):
        seg = res.results[c]["t"].astype(np.float64)  # [128, CH]
        t[c * R:(c + 1) * R] = seg.T.reshape(R)

    q64 = qv.astype(np.float64)
    field = t / TWOPI + 2.0 * SELF_C * q64
    pot = float((q64 * t).sum() / (2.0 * TWOPI) + SELF_C * (q64 * q64).sum())
    out = np.empty(N + 1, np.float32)
    out[0] = pot
    out[1:] = field.astype(np.float32)
    return out


# revision 10
# speedup vs baseline: 2.8080x; 2.8080x over previous
"""Trainium2 Bass kernel for the aperiodic real-space Ewald sum (N=4096).

Math: with w_ij = erf(d_ij/sqrt(2)) / (d_ij + eps) (symmetric),
    t_i   = sum_j q_j w_ij
    field = t/(2*pi) + 2*SELF_C*q
    pot   = (q . t)/(4*pi) + SELF_C*sum(q^2)

Sharding: core c owns rows [c*512, (c+1)*512). Each 128-row chunk computes
its [128, 4096] block of w against ALL columns j, multiplies by q_j along
the free axis, and reduces along the free axis (DVE accum_out) -- giving
t for its own rows directly. No partition reduction and no cross-core
combining; the host concatenates row segments.

d^2 = s_i + s_j - 2 x_i.x_j is computed as a K=13 bf16 matmul using a
double-bf16 (hi+lo) decomposition of both factors, which matches fp32
accuracy (~1e-3 abs) at bf16 PE speed (1 cycle/row vs 4 for fp32):
  rows 0-2: ahi.bhi, 3-5: ahi.blo, 6-8: alo.bhi  (a=-2x_i, b=x_j)
  rows 9,10: (shi+slo)_i * 1,  rows 11,12: 1 * (shi+slo)_j
plus a bf16 BIG*I matmul on the diagonal block so the self-pair lands at
d=2^20 (w_ii ~ 2^-20 ~ 0).

Per chunk: ACT: d = Sqrt(p) [, r = AbsRsqrt(p)]; conv = Erf(d/sqrt(2))
           DVE: [r = reciprocal_approx_fast(d);] u = conv*qb;
                z = u*r (or u/d) with accum_out -> tloc[:, chunk]
"""
import sys

sys.path.insert(0, "/opt/trn_rl_repo")

import numpy as np
import ml_dtypes

import concourse.bass as bass
import concourse.tile as tile
from concourse import bacc, mybir
from concourse.bass_utils import run_bass_kernel_spmd

N = 4096
NCORES = 8
R = N // NCORES          # rows per core
CH = R // 128            # 128-row chunks per core
K = 13                   # contraction depth of the double-bf16 d^2 matmul
SIGMA = 1.0
TWOPI = 2.0 * np.pi
SELF_C = 1.0 / (SIGMA * TWOPI**1.5)
INV_SQRT2 = float(1.0 / np.sqrt(2.0))
BIG = float(2.0**40)
DCLAMP = 0.02            # d~2 = d^2 + DCLAMP: keeps Sqrt off negative inputs
                         # (double-bf16 d^2 err ~ +-0.009, true min d^2 ~ 2e-6;
                         # w(sqrt(d^2+c)) - w(d) <= ~3e-4 since w is flat at 0)

_nc_cache = None


def _build_nc(loop_n=None, unroll=1, *, r_mode="act"):
    """r_mode: 'act'  -> r = AbsRsqrt(p) on ACT (3 ACT passes, 2 DVE)
               'recip'-> r = reciprocal_approx_fast(d) on DVE (2 ACT, 3 DVE)
               'div'  -> z = u / d on DVE (2 ACT, 2 DVE)"""
    nc = bacc.Bacc("TRN2", target_bir_lowering=False, debug=False,
                   num_devices=NCORES)
    f32 = mybir.dt.float32
    bf16 = mybir.dt.bfloat16
    E = mybir.ActivationFunctionType
    mult = mybir.AluOpType.mult
    div = mybir.AluOpType.divide

    aug_d = nc.dram_tensor("aug", [K, R + N], bf16, kind="ExternalInput").ap()
    qb_d = nc.dram_tensor("qb", [128, N], bf16, kind="ExternalInput").ap()
    idn_d = nc.dram_tensor("idn", [128, 256], bf16, kind="ExternalInput").ap()
    t_d = nc.dram_tensor("t", [128, CH], f32, kind="ExternalOutput").ap()

    with tile.TileContext(nc) as tc:
        with (
            tc.tile_pool(name="sbin", bufs=1) as sbin,
            tc.tile_pool(name="psum", bufs=1, space="PSUM") as psum,
            tc.tile_pool(name="sbd", bufs=2) as sbd,
            tc.tile_pool(name="sbr", bufs=2) as sbr,
            tc.tile_pool(name="sbc", bufs=2) as sbc,
            tc.tile_pool(name="sbu", bufs=1) as sbu,
            tc.tile_pool(name="sbz", bufs=1) as sbz,
            tc.tile_pool(name="sbt", bufs=2) as sbt,
        ):
            aug = sbin.tile([K, R + N], bf16, tag="aug")
            nc.sync.dma_start(aug[:], aug_d[:])
            qb = sbin.tile([128, N], bf16, tag="qb")
            nc.sync.dma_start(qb[:], qb_d[:])
            idn = sbin.tile([128, 256], bf16, tag="idn")
            nc.sync.dma_start(idn[:], idn_d[:])
            cb = sbin.tile([128, 1], f32, tag="cb")
            nc.gpsimd.memset(cb[:], DCLAMP)

            def body():
                tloc = sbt.tile([128, CH], f32, tag="tloc")
                for ic in range(CH):
                    p = psum.tile([128, N], f32, tag="p")
                    for k4 in range(N // 512):
                        nc.tensor.matmul(
                            p[:, k4 * 512:(k4 + 1) * 512],
                            aug[:, ic * 128:(ic + 1) * 128],
                            aug[:, R + k4 * 512:R + (k4 + 1) * 512],
                            start=True, stop=not (k4 == 0))
                        if k4 == 0:
                            nc.tensor.matmul(
                                p[:, ic * 128:(ic + 1) * 128],
                                idn[:, 0:128], idn[:, 128:256],
                                start=False, stop=True)
                    d = sbd.tile([128, N], f32, tag="d")
                    nc.scalar.activation(d[:], p[:], E.Sqrt, bias=cb[:])
                    if r_mode == "act":
                        r = sbr.tile([128, N], bf16, tag="r")
                        nc.scalar.activation(r[:], p[:],
                                             E.Abs_reciprocal_sqrt,
                                             bias=cb[:])
                    conv = sbc.tile([128, N], bf16, tag="conv")
                    nc.scalar.activation(conv[:], d[:], E.Erf,
                                         scale=INV_SQRT2)
                    if r_mode == "recip":
                        r = sbr.tile([128, N], f32, tag="r")
                        nc.vector.reciprocal_approx_fast(r[:], d[:])
                    u = sbu.tile([128, N], bf16, tag="u")
                    nc.vector.tensor_tensor(u[:], conv[:], qb[:], op=mult)
                    z = sbz.tile([128, N], bf16, tag="z")
                    if r_mode == "div":
                        nc.vector.scalar_tensor_tensor(
                            out=z[:], in0=u[:], scalar=1.0, in1=d[:],
                            op0=mult, op1=div,
                            accum_out=tloc[:, ic:ic + 1])
                    else:
                        nc.vector.scalar_tensor_tensor(
                            out=z[:], in0=u[:], scalar=1.0, in1=r[:],
                            op0=mult, op1=mult,
                            accum_out=tloc[:, ic:ic + 1])
                nc.sync.dma_start(t_d[:], tloc[:])

            if loop_n is not None:
                with tc.For_i(0, loop_n, 1):
                    for _ in range(unroll):
                        body()
            else:
                body()
    nc.compile()
    return nc


def _hi_lo(v):
    hi = v.astype(ml_dtypes.bfloat16)
    lo = (v - hi.astype(np.float32)).astype(ml_dtypes.bfloat16)
    return hi, lo


def _prep_inputs(positions, q):
    pos = np.ascontiguousarray(np.asarray(positions, dtype=np.float32))
    qv = np.asarray(q, dtype=np.float32).reshape(-1)
    s = (pos * pos).sum(axis=1, dtype=np.float32)

    bhi, blo = _hi_lo(pos.T)              # [3, N]
    shi, slo = _hi_lo(s)                  # [N]
    ahi, alo = _hi_lo(-2.0 * pos.T)       # [3, N]

    rhs_all = np.zeros((K, N), ml_dtypes.bfloat16)
    rhs_all[0:3] = bhi
    rhs_all[3:6] = blo
    rhs_all[6:9] = bhi
    rhs_all[9] = 1.0
    rhs_all[10] = 1.0
    rhs_all[11] = shi
    rhs_all[12] = slo

    lhs_all = np.zeros((K, N), ml_dtypes.bfloat16)
    lhs_all[0:3] = ahi
    lhs_all[3:6] = ahi
    lhs_all[6:9] = alo
    lhs_all[9] = shi
    lhs_all[10] = slo
    lhs_all[11] = 1.0
    lhs_all[12] = 1.0

    idn = np.concatenate([np.eye(128), np.eye(128) * BIG],
                         axis=1).astype(ml_dtypes.bfloat16)

    in_maps = []
    for c in range(NCORES):
        blk = slice(c * R, (c + 1) * R)
        aug = np.empty((K, R + N), ml_dtypes.bfloat16)
        aug[:, 0:R] = lhs_all[:, blk]
        aug[:, R:] = np.roll(rhs_all, -c * R, axis=1)
        qb = np.ascontiguousarray(
            np.broadcast_to(np.roll(qv, -c * R)[None, :], (128, N))
        ).astype(ml_dtypes.bfloat16)
        in_maps.append({"aug": aug, "qb": qb, "idn": idn})
    return in_maps, qv


def kernel(positions, q):
    global _nc_cache
    if _nc_cache is None:
        _nc_cache = _build_nc()
    nc = _nc_cache

    in_maps, qv = _prep_inputs(positions, q)
    res = run_bass_kernel_spmd(nc, in_maps, core_ids=list(range(NCORES)))

    t = np.empty(N, np.float64)
    for c in range(NCORES):
        seg = res.results[c]["t"].astype(np.float64)  # [128, CH]
        t[c * R:(c + 1) * R] = seg.T.reshape(R)

    q64 = qv.astype(np.float64)
    field = t / TWOPI + 2.0 * SELF_C * q64
    pot = float((q64 * t).sum() / (2.0 * TWOPI) + SELF_C * (q64 * q64).sum())
    out = np.empty(N + 1, np.float32)
    out[0] = pot
    out[1:] = field.astype(np.float32)
    return out


# revision 15
# speedup vs baseline: 6.2534x; 2.2270x over previous
"""Trainium2 Bass kernel for the aperiodic real-space Ewald sum (N=4096).

Math: with w_ij = erf(d_ij/sqrt(2)) / (d_ij + eps) (symmetric),
    t_i   = sum_j q_j w_ij
    field = t/(2*pi) + 2*SELF_C*q
    pot   = (q . t)/(4*pi) + SELF_C*sum(q^2)

Symmetric half-work sharding: core c owns rows [c*512, (c+1)*512). Each
128-row chunk computes w only for column offsets delta = (j - i) mod N in
[1, 1919] (a [128, 2048] rectangle in rolled column coords; BIG is added
to the delta <= 0 lower triangle and delta >= 1920 upper triangle via two
small accumulated matmuls, so those pairs vanish through erf(d)/d at
d ~ 2^20). Each block is then reduced BOTH ways:
  - free-axis DVE accum of (conv*r)*qb -> t contributions for its rows
  - K=128 PE matmul qc^T @ z2         -> t contributions for its columns
so each unordered pair is computed exactly once on device. The host adds
the exact f64 contribution of the uncovered offset band delta in
[1920, 2176] (~1M ordered pairs, a few ms of numpy) and assembles t.

d^2 = s_i + s_j - 2 x_i.x_j is a K=13 bf16 matmul via double-bf16 (hi+lo)
splits of both factors -- fp32-class accuracy (~1e-3 abs) at bf16 PE rate.
Per chunk: ACT: d = Sqrt(p+.02); r = AbsRsqrt(p+.02) bf16; conv = Erf(c*d) bf16
           DVE: z2 = conv*r bf16; u = z2*qb with accum_out -> tloc[:, chunk]
           PE : tcp[1, 2048] = qc^T @ z2 -> per-chunk DMA (from PSUM)
The PE stream is software-pipelined: chunk ic+1's d^2 matmuls are emitted
before chunk ic's tcp matmuls so the PE never stalls behind the DVE.
"""
import sys

sys.path.insert(0, "/opt/trn_rl_repo")

import numpy as np
import ml_dtypes

import concourse.bass as bass
import concourse.tile as tile
from concourse import bacc, mybir
from concourse.bass_utils import run_bass_kernel_spmd

try:
    from scipy.special import erf as _erf
except ImportError:
    import math
    _erf = np.vectorize(math.erf)

N = 4096
NCORES = 8
R = N // NCORES          # rows per core
CH = R // 128            # 128-row chunks per core
K = 13                   # contraction depth of the double-bf16 d^2 matmul
W = 2048                 # rectangle width (device covers delta in [1, 1919])
DMAX = 1920              # first host-handled offset
DBAND = 257              # host band: delta in [1920, 2176]
COLS = (CH - 1) * 128 + W   # rhs columns needed locally (2432)
SIGMA = 1.0
TWOPI = 2.0 * np.pi
SELF_C = 1.0 / (SIGMA * TWOPI**1.5)
INV_SQRT2 = float(1.0 / np.sqrt(2.0))
BIG = float(2.0**40)
DCLAMP = 0.02            # keeps Sqrt input positive vs d^2 err ~ +-0.009

_nc_cache = None


def _build_nc(loop_n=None, unroll=1):
    nc = bacc.Bacc("TRN2", target_bir_lowering=False, debug=False,
                   num_devices=NCORES)
    f32 = mybir.dt.float32
    bf16 = mybir.dt.bfloat16
    E = mybir.ActivationFunctionType
    mult = mybir.AluOpType.mult

    aug_d = nc.dram_tensor("aug", [K, R + COLS], bf16,
                           kind="ExternalInput").ap()
    qb_d = nc.dram_tensor("qb", [128, COLS], bf16, kind="ExternalInput").ap()
    msk_d = nc.dram_tensor("msk", [128, 384], bf16, kind="ExternalInput").ap()
    qc_d = nc.dram_tensor("qc", [128, CH], bf16, kind="ExternalInput").ap()
    t_d = nc.dram_tensor("t", [128, CH], f32, kind="ExternalOutput").ap()
    tc_d = nc.dram_tensor("tc", [CH, W], f32, kind="ExternalOutput").ap()

    with tile.TileContext(nc) as tc:
        with (
            tc.tile_pool(name="sbin", bufs=1) as sbin,
            tc.tile_pool(name="psum", bufs=1, space="PSUM") as psum,
            tc.tile_pool(name="psumc", bufs=1, space="PSUM") as psumc,
            tc.tile_pool(name="sbd", bufs=2) as sbd,
            tc.tile_pool(name="sbr", bufs=2) as sbr,
            tc.tile_pool(name="sbc", bufs=2) as sbc,
            tc.tile_pool(name="sbz", bufs=2) as sbz,
            tc.tile_pool(name="sbu", bufs=2) as sbu,
            tc.tile_pool(name="sbt", bufs=2) as sbt,
            tc.tile_pool(name="sbtc", bufs=2) as sbtc,
        ):
            aug = sbin.tile([K, R + COLS], bf16, tag="aug")
            nc.sync.dma_start(aug[:], aug_d[:])
            qb = sbin.tile([128, COLS], bf16, tag="qb")
            nc.sync.dma_start(qb[:], qb_d[:])
            msk = sbin.tile([128, 384], bf16, tag="msk")
            nc.sync.dma_start(msk[:], msk_d[:])
            qc = sbin.tile([128, CH], bf16, tag="qc")
            nc.sync.dma_start(qc[:], qc_d[:])
            cb = sbin.tile([128, 1], f32, tag="cb")
            nc.gpsimd.memset(cb[:], DCLAMP)

            def emit_d2(ic):
                p = psum.tile([128, W], f32, tag="p", name=f"p{ic}")
                base = R + ic * 128
                for k4 in range(W // 512):
                    two = k4 == 0 or k4 == 3
                    nc.tensor.matmul(
                        p[:, k4 * 512:(k4 + 1) * 512],
                        aug[:, ic * 128:(ic + 1) * 128],
                        aug[:, base + k4 * 512:base + (k4 + 1) * 512],
                        start=True, stop=not two)
                    if k4 == 0:
                        nc.tensor.matmul(p[:, 0:128], msk[:, 0:128],
                                         msk[:, 128:256],
                                         start=False, stop=True)
                    elif k4 == 3:
                        nc.tensor.matmul(p[:, 1920:2048], msk[:, 0:128],
                                         msk[:, 256:384],
                                         start=False, stop=True)
                return p

            def body():
                tloc = sbt.tile([128, CH], f32, tag="tloc")
                p = emit_d2(0)
                for ic in range(CH):
                    d = sbd.tile([128, W], f32, tag="d")
                    nc.scalar.activation(d[:], p[:], E.Sqrt, bias=cb[:])
                    r = sbr.tile([128, W], bf16, tag="r")
                    nc.scalar.activation(r[:], p[:], E.Abs_reciprocal_sqrt,
                                         bias=cb[:])
                    conv = sbc.tile([128, W], bf16, tag="conv")
                    nc.scalar.activation(conv[:], d[:], E.Erf,
                                         scale=INV_SQRT2)
                    z2 = sbz.tile([128, W], bf16, tag="z2")
                    nc.vector.tensor_tensor(z2[:], conv[:], r[:], op=mult)
                    u = sbu.tile([128, W], bf16, tag="u")
                    nc.vector.scalar_tensor_tensor(
                        out=u[:], in0=z2[:], scalar=1.0,
                        in1=qb[:, ic * 128:ic * 128 + W],
                        op0=mult, op1=mult,
                        accum_out=tloc[:, ic:ic + 1])
                    if ic + 1 < CH:
                        p = emit_d2(ic + 1)
                    tcp = psumc.tile([1, W], f32, tag="tcp")
                    for k4 in range(W // 512):
                        nc.tensor.matmul(
                            tcp[:, k4 * 512:(k4 + 1) * 512],
                            qc[:, ic:ic + 1],
                            z2[:, k4 * 512:(k4 + 1) * 512],
                            start=True, stop=True)
                    tcs = sbtc.tile([1, W], f32, tag=f"tcs{ic}",
                                    name=f"tcs{ic}")
                    nc.vector.tensor_scalar_mul(tcs[0:1, :],
                                                tcp[0:1, :], 1.0)
                    nc.sync.dma_start(tc_d[ic:ic + 1, :], tcs[0:1, :])
                nc.sync.dma_start(t_d[:], tloc[:])

            if loop_n is not None:
                with tc.For_i(0, loop_n, 1):
                    for _ in range(unroll):
                        body()
            else:
                body()
    nc.compile()
    return nc


def _hi_lo(v):
    hi = v.astype(ml_dtypes.bfloat16)
    lo = (v - hi.astype(np.float32)).astype(ml_dtypes.bfloat16)
    return hi, lo


def _prep_inputs(positions, q):
    pos = np.ascontiguousarray(np.asarray(positions, dtype=np.float32))
    qv = np.asarray(q, dtype=np.float32).reshape(-1)
    s = (pos * pos).sum(axis=1, dtype=np.float32)

    bhi, blo = _hi_lo(pos.T)              # [3, N]
    shi, slo = _hi_lo(s)                  # [N]
    ahi, alo = _hi_lo(-2.0 * pos.T)       # [3, N]

    rhs_all = np.zeros((K, N), ml_dtypes.bfloat16)
    rhs_all[0:3] = bhi
    rhs_all[3:6] = blo
    rhs_all[6:9] = bhi
    rhs_all[9] = 1.0
    rhs_all[10] = 1.0
    rhs_all[11] = shi
    rhs_all[12] = slo

    lhs_all = np.zeros((K, N), ml_dtypes.bfloat16)
    lhs_all[0:3] = ahi
    lhs_all[3:6] = ahi
    lhs_all[6:9] = alo
    lhs_all[9] = shi
    lhs_all[10] = slo
    lhs_all[11] = 1.0
    lhs_all[12] = 1.0

    # BIG masks accumulated onto d^2 (identity lhsT = the mask itself as
    # rhs): LT[p, l0] = BIG iff l0 <= p (delta <= 0, incl. self-pair);
    # UT[p, m] = BIG iff m >= p (delta >= 1920).
    pp, ll = np.meshgrid(np.arange(128), np.arange(128), indexing="ij")
    msk = np.zeros((128, 384), np.float32)
    msk[:, 0:128] = np.eye(128)
    msk[:, 128:256] = (ll <= pp) * BIG
    msk[:, 256:384] = (ll >= pp) * BIG
    msk = msk.astype(ml_dtypes.bfloat16)

    in_maps = []
    for c in range(NCORES):
        blk = slice(c * R, (c + 1) * R)
        idx = (np.arange(COLS) + c * R) % N
        aug = np.empty((K, R + COLS), ml_dtypes.bfloat16)
        aug[:, 0:R] = lhs_all[:, blk]
        aug[:, R:] = rhs_all[:, idx]
        qb = np.ascontiguousarray(
            np.broadcast_to(qv[idx][None, :], (128, COLS))
        ).astype(ml_dtypes.bfloat16)
        qc = np.ascontiguousarray(
            qv[blk].reshape(CH, 128).T).astype(ml_dtypes.bfloat16)
        in_maps.append({"aug": aug, "qb": qb, "msk": msk, "qc": qc})
    return in_maps, qv


def _band_correction(pos64, q64):
    """Exact f64 t contributions for offsets delta in [1920, 2176]."""
    t = np.zeros(N)
    deltas = np.arange(DMAX, DMAX + DBAND)
    i_idx = np.arange(N)[:, None]
    for d0 in range(0, DBAND, 64):
        ds = deltas[d0:d0 + 64]
        j = (i_idx + ds[None, :]) % N                       # [N, nb]
        diff = pos64[j] - pos64[:, None, :]                 # [N, nb, 3]
        dist = np.sqrt((diff**2).sum(-1))
        w = _erf(dist / np.sqrt(2.0)) / (dist + 1e-6)
        t += (q64[j] * w).sum(1)
    return t


def kernel(positions, q):
    global _nc_cache
    if _nc_cache is None:
        _nc_cache = _build_nc()
    nc = _nc_cache

    in_maps, qv = _prep_inputs(positions, q)
    res = run_bass_kernel_spmd(nc, in_maps, core_ids=list(range(NCORES)))

    t = np.zeros(N, np.float64)
    for c in range(NCORES):
        seg = res.results[c]["t"].astype(np.float64)   # [128, CH] row part
        t[c * R:(c + 1) * R] += seg.T.reshape(R)
        tcol = res.results[c]["tc"].astype(np.float64)  # [CH, W] col part
        for ic in range(CH):
            idx = (np.arange(W) + c * R + ic * 128) % N
            np.add.at(t, idx, tcol[ic])

    pos64 = np.asarray(positions, dtype=np.float64)
    q64 = qv.astype(np.float64)
    t += _band_correction(pos64, q64)

    field = t / TWOPI + 2.0 * SELF_C * q64
    pot = float((q64 * t).sum() / (2.0 * TWOPI) + SELF_C * (q64 * q64).sum())
    out = np.empty(N + 1, np.float32)
    out[0] = pot
    out[1:] = field.astype(np.float32)
    return out


# revision 17
# speedup vs baseline: 6.5718x; 1.0509x over previous
"""Trainium2 Bass kernel for the aperiodic real-space Ewald sum (N=4096).

Math: with w_ij = erf(d_ij/sqrt(2)) / (d_ij + eps) (symmetric),
    t_i   = sum_j q_j w_ij
    field = t/(2*pi) + 2*SELF_C*q
    pot   = (q . t)/(4*pi) + SELF_C*sum(q^2)

Symmetric half-work sharding: core c owns rows [c*512, (c+1)*512). Each
128-row chunk computes w only for column offsets delta = (j - i) mod N in
[1, 1919] (a [128, 2048] rectangle in rolled column coords; BIG is added
to the delta <= 0 lower triangle and delta >= 1920 upper triangle via two
small accumulated matmuls, so those pairs vanish through erf(d)/d at
d ~ 2^20). Each block is then reduced BOTH ways:
  - free-axis DVE accum of (conv*r)*qb -> t contributions for its rows
  - K=128 PE matmul qc^T @ z2         -> t contributions for its columns
so each unordered pair is computed exactly once on device. The host adds
the exact f64 contribution of the uncovered offset band delta in
[1920, 2176] (~1M ordered pairs, a few ms of numpy) and assembles t.

d^2 = s_i + s_j - 2 x_i.x_j is a K=13 bf16 matmul via double-bf16 (hi+lo)
splits of both factors -- fp32-class accuracy (~1e-3 abs) at bf16 PE rate.
Per chunk: ACT: d = Sqrt(p+.02); r = AbsRsqrt(p+.02) bf16; conv = Erf(c*d) bf16
           DVE: z2 = conv*r bf16; u = z2*qb with accum_out -> tloc[:, chunk]
           PE : tcp[1, 2048] = qc^T @ z2 -> per-chunk DMA (from PSUM)
The PE stream is software-pipelined: chunk ic+1's d^2 matmuls are emitted
before chunk ic's tcp matmuls so the PE never stalls behind the DVE.
"""
import sys

sys.path.insert(0, "/opt/trn_rl_repo")

import numpy as np
import ml_dtypes

import concourse.bass as bass
import concourse.tile as tile
from concourse import bacc, mybir
from concourse.bass_utils import run_bass_kernel_spmd

try:
    from scipy.special import erf as _erf
except ImportError:
    import math
    _erf = np.vectorize(math.erf)

N = 4096
NCORES = 8
R = N // NCORES          # rows per core
CH = R // 128            # 128-row chunks per core
K = 13                   # contraction depth of the double-bf16 d^2 matmul
W = 2048                 # rectangle width (device covers delta in [1, 1919])
DMAX = 1920              # first host-handled offset
DBAND = 257              # host band: delta in [1920, 2176]
COLS = (CH - 1) * 128 + W   # rhs columns needed locally (2432)
SIGMA = 1.0
TWOPI = 2.0 * np.pi
SELF_C = 1.0 / (SIGMA * TWOPI**1.5)
INV_SQRT2 = float(1.0 / np.sqrt(2.0))
BIG = float(2.0**40)
DCLAMP = 0.02            # keeps Sqrt input positive vs d^2 err ~ +-0.009

_nc_cache = None


def _build_nc(loop_n=None, unroll=1):
    nc = bacc.Bacc("TRN2", target_bir_lowering=False, debug=False,
                   num_devices=NCORES)
    f32 = mybir.dt.float32
    bf16 = mybir.dt.bfloat16
    E = mybir.ActivationFunctionType
    mult = mybir.AluOpType.mult

    aug_d = nc.dram_tensor("aug", [K, R + COLS], bf16,
                           kind="ExternalInput").ap()
    qb_d = nc.dram_tensor("qb", [128, COLS], bf16, kind="ExternalInput").ap()
    msk_d = nc.dram_tensor("msk", [128, 384], bf16, kind="ExternalInput").ap()
    qc_d = nc.dram_tensor("qc", [128, CH], bf16, kind="ExternalInput").ap()
    t_d = nc.dram_tensor("t", [128, CH], f32, kind="ExternalOutput").ap()
    tc_d = nc.dram_tensor("tc", [CH, W], f32, kind="ExternalOutput").ap()

    with tile.TileContext(nc) as tc:
        with (
            tc.tile_pool(name="sbin", bufs=1) as sbin,
            tc.tile_pool(name="psum", bufs=1, space="PSUM") as psum,
            tc.tile_pool(name="psumc", bufs=1, space="PSUM") as psumc,
            tc.tile_pool(name="sbd", bufs=2) as sbd,
            tc.tile_pool(name="sbr", bufs=2) as sbr,
            tc.tile_pool(name="sbc", bufs=2) as sbc,
            tc.tile_pool(name="sbz", bufs=2) as sbz,
            tc.tile_pool(name="sbu", bufs=2) as sbu,
            tc.tile_pool(name="sbt", bufs=2) as sbt,
            tc.tile_pool(name="sbtc", bufs=2) as sbtc,
        ):
            aug = sbin.tile([K, R + COLS], bf16, tag="aug")
            nc.sync.dma_start(aug[:], aug_d[:])
            qb = sbin.tile([128, COLS], bf16, tag="qb")
            nc.sync.dma_start(qb[:], qb_d[:])
            msk = sbin.tile([128, 384], bf16, tag="msk")
            nc.sync.dma_start(msk[:], msk_d[:])
            qc = sbin.tile([128, CH], bf16, tag="qc")
            nc.sync.dma_start(qc[:], qc_d[:])
            cb = sbin.tile([128, 1], f32, tag="cb")
            nc.gpsimd.memset(cb[:], DCLAMP)

            def emit_d2(ic):
                p = psum.tile([128, W], f32, tag="p", name=f"p{ic}")
                base = R + ic * 128
                for k4 in range(W // 512):
                    two = k4 == 0 or k4 == 3
                    nc.tensor.matmul(
                        p[:, k4 * 512:(k4 + 1) * 512],
                        aug[:, ic * 128:(ic + 1) * 128],
                        aug[:, base + k4 * 512:base + (k4 + 1) * 512],
                        start=True, stop=not two)
                    if k4 == 0:
                        nc.tensor.matmul(p[:, 0:128], msk[:, 0:128],
                                         msk[:, 128:256],
                                         start=False, stop=True)
                    elif k4 == 3:
                        nc.tensor.matmul(p[:, 1920:2048], msk[:, 0:128],
                                         msk[:, 256:384],
                                         start=False, stop=True)
                return p

            def body():
                tloc = sbt.tile([128, CH], f32, tag="tloc")
                p = emit_d2(0)
                for ic in range(CH):
                    d = sbd.tile([128, W], f32, tag="d")
                    nc.scalar.activation(d[:], p[:], E.Sqrt, bias=cb[:])
                    r = sbr.tile([128, W], bf16, tag="r")
                    nc.scalar.activation(r[:], p[:], E.Abs_reciprocal_sqrt,
                                         bias=cb[:])
                    conv = sbc.tile([128, W], bf16, tag="conv")
                    nc.scalar.activation(conv[:], d[:], E.Erf,
                                         scale=INV_SQRT2)
                    z2 = sbz.tile([128, W], bf16, tag="z2")
                    nc.vector.tensor_tensor(z2[:], conv[:], r[:], op=mult)
                    u = sbu.tile([128, W], bf16, tag="u")
                    nc.vector.scalar_tensor_tensor(
                        out=u[:], in0=z2[:], scalar=1.0,
                        in1=qb[:, ic * 128:ic * 128 + W],
                        op0=mult, op1=mult,
                        accum_out=tloc[:, ic:ic + 1])
                    if ic + 1 < CH:
                        p = emit_d2(ic + 1)
                    tcp = psumc.tile([1, W], f32, tag="tcp")
                    for k4 in range(W // 512):
                        nc.tensor.matmul(
                            tcp[:, k4 * 512:(k4 + 1) * 512],
                            qc[:, ic:ic + 1],
                            z2[:, k4 * 512:(k4 + 1) * 512],
                            start=True, stop=True)
                    tcs = sbtc.tile([1, W], f32, tag=f"tcs{ic}",
                                    name=f"tcs{ic}")
                    nc.vector.tensor_scalar_mul(tcs[0:1, :],
                                                tcp[0:1, :], 1.0)
                    nc.sync.dma_start(tc_d[ic:ic + 1, :], tcs[0:1, :])
                nc.sync.dma_start(t_d[:], tloc[:])

            if loop_n is not None:
                with tc.For_i(0, loop_n, 1):
                    for _ in range(unroll):
                        body()
            else:
                body()
    nc.compile()
    return nc


def _hi_lo(v):
    hi = v.astype(ml_dtypes.bfloat16)
    lo = (v - hi.astype(np.float32)).astype(ml_dtypes.bfloat16)
    return hi, lo


def _prep_inputs(positions, q):
    pos = np.ascontiguousarray(np.asarray(positions, dtype=np.float32))
    qv = np.asarray(q, dtype=np.float32).reshape(-1)
    s = (pos * pos).sum(axis=1, dtype=np.float32)

    bhi, blo = _hi_lo(pos.T)              # [3, N]
    shi, slo = _hi_lo(s)                  # [N]
    ahi, alo = _hi_lo(-2.0 * pos.T)       # [3, N]

    rhs_all = np.zeros((K, N), ml_dtypes.bfloat16)
    rhs_all[0:3] = bhi
    rhs_all[3:6] = blo
    rhs_all[6:9] = bhi
    rhs_all[9] = 1.0
    rhs_all[10] = 1.0
    rhs_all[11] = shi
    rhs_all[12] = slo

    lhs_all = np.zeros((K, N), ml_dtypes.bfloat16)
    lhs_all[0:3] = ahi
    lhs_all[3:6] = ahi
    lhs_all[6:9] = alo
    lhs_all[9] = shi
    lhs_all[10] = slo
    lhs_all[11] = 1.0
    lhs_all[12] = 1.0

    # BIG masks accumulated onto d^2 (identity lhsT = the mask itself as
    # rhs): LT[p, l0] = BIG iff l0 <= p (delta <= 0, incl. self-pair);
    # UT[p, m] = BIG iff m >= p (delta >= 1920).
    pp, ll = np.meshgrid(np.arange(128), np.arange(128), indexing="ij")
    msk = np.zeros((128, 384), np.float32)
    msk[:, 0:128] = np.eye(128)
    msk[:, 128:256] = (ll <= pp) * BIG
    msk[:, 256:384] = (ll >= pp) * BIG
    msk = msk.astype(ml_dtypes.bfloat16)

    in_maps = []
    for c in range(NCORES):
        blk = slice(c * R, (c + 1) * R)
        idx = (np.arange(COLS) + c * R) % N
        aug = np.empty((K, R + COLS), ml_dtypes.bfloat16)
        aug[:, 0:R] = lhs_all[:, blk]
        aug[:, R:] = rhs_all[:, idx]
        qb = np.ascontiguousarray(
            np.broadcast_to(qv[idx][None, :], (128, COLS))
        ).astype(ml_dtypes.bfloat16)
        qc = np.ascontiguousarray(
            qv[blk].reshape(CH, 128).T).astype(ml_dtypes.bfloat16)
        in_maps.append({"aug": aug, "qb": qb, "msk": msk, "qc": qc})
    return in_maps, qv


def _band_correction(pos64, q64):
    """Exact f64 t contributions for offsets delta in [1920, 2176]."""
    t = np.zeros(N)
    deltas = np.arange(DMAX, DMAX + DBAND)
    i_idx = np.arange(N)[:, None]
    for d0 in range(0, DBAND, 64):
        ds = deltas[d0:d0 + 64]
        j = (i_idx + ds[None, :]) % N                       # [N, nb]
        diff = pos64[j] - pos64[:, None, :]                 # [N, nb, 3]
        dist = np.sqrt((diff**2).sum(-1))
        w = _erf(dist / np.sqrt(2.0)) / (dist + 1e-6)
        t += (q64[j] * w).sum(1)
    return t


def kernel(positions, q):
    global _nc_cache
    if _nc_cache is None:
        _nc_cache = _build_nc()
    nc = _nc_cache

    in_maps, qv = _prep_inputs(positions, q)
    res = run_bass_kernel_spmd(nc, in_maps, core_ids=list(range(NCORES)))

    t = np.zeros(N, np.float64)
    for c in range(NCORES):
        seg = res.results[c]["t"].astype(np.float64)   # [128, CH] row part
        t[c * R:(c + 1) * R] += seg.T.reshape(R)
        tcol = res.results[c]["tc"].astype(np.float64)  # [CH, W] col part
        for ic in range(CH):
            idx = (np.arange(W) + c * R + ic * 128) % N
            np.add.at(t, idx, tcol[ic])

    pos64 = np.asarray(positions, dtype=np.float64)
    q64 = qv.astype(np.float64)
    t += _band_correction(pos64, q64)

    field = t / TWOPI + 2.0 * SELF_C * q64
    pot = float((q64 * t).sum() / (2.0 * TWOPI) + SELF_C * (q64 * q64).sum())
    out = np.empty(N + 1, np.float32)
    out[0] = pot
    out[1:] = field.astype(np.float32)
    return out
